# revision 1
# baseline (speedup 1.0000x reference)
"""MetaQDA forward on 8 Trainium2 NeuronCores.

Math: the per-class covariance is sigma_c = coef * (B + U_c J U_c^T) with
B = L L^T + kap m^T m shared across classes, U_c = [Xg_c^T, mu_c] (D x 17),
J = diag(1,...,1, -(kap+S)).  Woodbury + the matrix determinant lemma turn
the C=64 dense 512x512 inversions / logdets into rank-17 corrections, and
the Mahalanobis logits become one dense [Q,512] x [512,2752] GEMM plus a
small elementwise epilogue.  Queries are sharded across the 8 cores; the
class statistics (tiny after the reduction) are replicated.
"""
import math
from contextlib import ExitStack

import numpy as np

import concourse.bass as bass
import concourse.tile as tile
from concourse import bacc, mybir
from concourse.bass_utils import run_bass_kernel_spmd

REG = 0.1
D = 512
C = 64
Q = 2048
N_CORES = 8
QC = Q // N_CORES          # 256 queries per core
P = 128                    # partitions
R = None                   # rank per class (S+1), set in prep
F32 = mybir.dt.float32


# ---------------------------------------------------------------- host prep
def _prep(X_support, labels, X_query, m, kappa, nu, triu_diag, triu_lower,
          n_classes):
    f = np.float64
    Xs = np.asarray(X_support, f)
    Nn, Dd = Xs.shape
    Cc = int(n_classes)
    S = Nn // Cc
    r = S + 1
    m_ = np.asarray(m, f).reshape(1, Dd)
    kap = abs(float(kappa)) + 1e-6
    nu_ = max(float(nu), Dd - 1 + 1e-6)

    order = np.argsort(np.asarray(labels), kind="stable")
    Xg = Xs[order].reshape(Cc, S, Dd)
    mu = (kap / (kap + S)) * m_ + (S / (kap + S)) * Xg.mean(axis=1)  # [C,D]

    Lmask = np.tril(np.ones((Dd, Dd), f), -1)
    L = np.diag(np.abs(np.asarray(triu_diag, f))) + np.asarray(triu_lower, f) * Lmask
    B = L @ L.T + kap * (m_.T @ m_)
    coef = (kap + S + 1.0) / ((nu_ + S - Dd + 1.0) * (kap + S))
    alpha = (1.0 - REG) / coef
    common = nu_ + S + 1.0 - Dd
    beta = 0.5 * (common + Dd)

    Binv = np.linalg.inv(B)
    _, ldB = np.linalg.slogdet(B)

    U = np.concatenate([Xg.transpose(0, 2, 1), mu[:, :, None]], axis=2)  # [C,D,r]
    V = np.matmul(Binv, U)                                   # [C,D,r]
    Jinv = np.diag(np.concatenate([np.ones(S), [-1.0 / (kap + S)]]))
    M = Jinv[None] + np.swapaxes(U, 1, 2) @ V                # [C,r,r]
    Ninv = np.linalg.inv(M)
    _, ldM = np.linalg.slogdet(M)

    muB = mu @ Binv                                          # [C,D]
    b = np.einsum("cdr,cd->cr", V, mu)                       # [C,r]
    kq = np.einsum("cd,cd->c", mu, muB)
    VN = V @ Ninv                                            # [C,D,r]
    VNb = np.einsum("cdr,cr->cd", VN, b)
    Nb = np.einsum("crs,cs->cr", Ninv, b)

    linW = (-2.0 * alpha * (muB - VNb) - 2.0 * REG * mu).T   # [D,C]
    cc = (alpha * (kq - np.einsum("cr,cr->c", b, Nb))
          + REG * np.einsum("cd,cd->c", mu, mu) + common)    # [C]

    logdet = Dd * np.log(coef) + ldB + np.log(kap + S) + ldM
    bias = (math.lgamma(0.5 * (common + Dd)) - math.lgamma(0.5 * common)
            - 0.5 * Dd * np.log(common) - 0.5 * logdet)
    gam = bias + beta * np.log(common)                       # [C]

    V_all = V.transpose(1, 0, 2).reshape(Dd, Cc * r)
    E_all = (-alpha * VN).transpose(1, 0, 2).reshape(Dd, Cc * r)
    Wcat = np.concatenate([V_all, E_all, linW, Binv], axis=1)  # [D, 2*C*r+C+D]
    return (Wcat.astype(np.float32), cc.astype(np.float32),
            gam.astype(np.float32), float(alpha), float(beta), r)


# ---------------------------------------------------------------- device IR
_CACHE = {}


def _build(alpha, beta, r):
    NW = 2 * C * r + C + D       # 2752 wcat columns
    WX = QC + NW                 # xqt cols then wcat cols, fused
    nc = bacc.Bacc("TRN2", target_bir_lowering=False, debug=False,
                   num_devices=N_CORES)
    F32R = mybir.dt.float32r
    xq = nc.declare_dram_parameter("xq", [QC, D], F32, isOutput=False)
    wx = nc.declare_dram_parameter("wx", [D, WX], F32R, isOutput=False)
    ccg = nc.declare_dram_parameter("ccg", [P, C], F32, isOutput=False)
    gam = nc.declare_dram_parameter("gam", [P, C], F32, isOutput=False)
    out = nc.declare_dram_parameter("out", [QC, C], F32, isOutput=True)

    KT = D // P                  # 4 k-steps
    QT = QC // P                 # 2 query tiles
    chunks = []
    n0 = 0
    while n0 < NW:
        nw = min(512, NW - n0)
        chunks.append((n0, nw))
        n0 += nw

    wv = wx[:].rearrange("(k p) n -> k p n", p=P)
    xv = xq[:].rearrange("(t p) d -> t p d", p=P)
    ov = out[:].rearrange("(t p) c -> t p c", p=P)

    with tile.TileContext(nc) as tc, ExitStack() as ctx:
        wpool = ctx.enter_context(tc.tile_pool(name="w", bufs=1))
        iopool = ctx.enter_context(tc.tile_pool(name="io", bufs=1))
        opool = ctx.enter_context(tc.tile_pool(name="o", bufs=2))
        spool = ctx.enter_context(tc.tile_pool(name="s", bufs=2))
        pspool = ctx.enter_context(
            tc.tile_pool(name="ps", bufs=4, space="PSUM"))

        w_sb = []
        for k in range(KT):
            wt = wpool.tile([P, WX], F32R, tag=f"w{k}")
            nc.sync.dma_start(wt[:], wv[k])
            w_sb.append(wt)
        cc_sb = iopool.tile([P, C], F32, tag="cc")
        nc.sync.dma_start(cc_sb[:], ccg[:])
        gm_sb = iopool.tile([P, C], F32, tag="gm")
        nc.sync.dma_start(gm_sb[:], gam[:])

        for t in range(QT):
            xq_sb = spool.tile([P, D], F32, tag="xq")
            nc.sync.dma_start(xq_sb[:], xv[t])

            osb = opool.tile([P, NW], F32, tag="osb")
            for (n0, nw) in chunks:
                ps = pspool.tile([P, nw], F32, tag="ps")
                for k in range(KT):
                    nc.tensor.matmul(
                        ps[:], w_sb[k][:, t * P:(t + 1) * P],
                        w_sb[k][:, QC + n0:QC + n0 + nw],
                        start=(k == 0), stop=(k == KT - 1))
                nc.vector.tensor_copy(osb[:, n0:n0 + nw], ps[:])

            # acc = alpha * x^T Binv x + REG * x^T x        [P,1]
            scr = spool.tile([P, D], F32, tag="scr")
            s2 = spool.tile([P, 1], F32, tag="s2")
            nc.scalar.activation(
                scr[:], xq_sb[:], mybir.ActivationFunctionType.Square,
                scale=float(math.sqrt(REG)), accum_out=s2[:])
            scr2 = spool.tile([P, D], F32, tag="scr2")
            g0 = spool.tile([P, 1], F32, tag="g0")
            nc.vector.tensor_mul(scr2[:], osb[:, 2 * C * r + C:NW], xq_sb[:])
            nc.vector.tensor_reduce(
                out=g0[:], in_=scr2[:], axis=mybir.AxisListType.X,
                op=mybir.AluOpType.add)
            acc = spool.tile([P, 1], F32, tag="acc")
            nc.vector.tensor_scalar(
                out=acc[:], in0=g0[:], scalar1=alpha, scalar2=s2[:],
                op0=mybir.AluOpType.mult, op1=mybir.AluOpType.add)

            # seg[q,c] = sum_i A1[q,c,i] * A2[q,c,i]  (the -alpha x^T VNV^T x term)
            prod = spool.tile([P, C * r], F32, tag="prod")
            nc.vector.tensor_mul(prod[:], osb[:, 0:C * r], osb[:, C * r:2 * C * r])
            seg = spool.tile([P, C], F32, tag="seg")
            nc.vector.tensor_reduce(
                out=seg[:], in_=prod[:].rearrange("p (c r) -> p c r", r=r),
                axis=mybir.AxisListType.X, op=mybir.AluOpType.add)

            # tdist = common + dist; logits = gam - beta * ln(tdist)
            td = spool.tile([P, C], F32, tag="td")
            nc.vector.tensor_add(td[:], seg[:], cc_sb[:])
            nc.vector.tensor_add(td[:], td[:], osb[:, 2 * C * r:2 * C * r + C])
            nc.vector.tensor_scalar_add(td[:], td[:], acc[:])
            lg = spool.tile([P, C], F32, tag="lg")
            nc.scalar.activation(lg[:], td[:], mybir.ActivationFunctionType.Ln)
            res = spool.tile([P, C], F32, tag="res")
            nc.vector.tensor_scalar_mul(res[:], lg[:], -beta)
            nc.vector.tensor_add(res[:], res[:], gm_sb[:])
            nc.sync.dma_start(ov[t], res[:])

    nc.compile()
    return nc


def _get_nc(alpha, beta, r):
    key = (round(alpha, 9), round(beta, 9), r)
    if key not in _CACHE:
        _CACHE.clear()
        _CACHE[key] = _build(alpha, beta, r)
    return _CACHE[key]


def kernel(X_support, labels, X_query, m, kappa, nu, triu_diag, triu_lower,
           n_classes):
    Wcat, cc, gam, alpha, beta, r = _prep(
        X_support, labels, X_query, m, kappa, nu, triu_diag, triu_lower,
        n_classes)
    ccg = np.ascontiguousarray(np.broadcast_to(cc[None, :], (P, C)))
    gamg = np.ascontiguousarray(np.broadcast_to(gam[None, :], (P, C)))

    nc = _get_nc(alpha, beta, r)

    Xq = np.asarray(X_query, np.float32)
    in_maps = []
    for i in range(N_CORES):
        sl = np.ascontiguousarray(Xq[i * QC:(i + 1) * QC])
        wxc = np.concatenate([sl.T, Wcat], axis=1)
        in_maps.append({
            "xq": sl,
            "wx": np.ascontiguousarray(wxc),
            "ccg": ccg,
            "gam": gamg,
        })
    res = run_bass_kernel_spmd(nc, in_maps, list(range(N_CORES)))
    return np.concatenate([res.results[i]["out"] for i in range(N_CORES)],
                          axis=0)



# revision 7
# speedup vs baseline: 1.5741x; 1.5741x over previous
"""MetaQDA forward on 8 Trainium2 NeuronCores.

Math: sigma_inv_reg = alpha*(Binv - V Ninv V^T) + REG*I with
B = L L^T + kap m^T m shared across classes (Woodbury over the rank-17
per-class update U_c = [Xg_c^T, mu_c]).  Per class, M_c = Jinv + U^T Binv U
is symmetric with EXACTLY one negative eigenvalue (det J < 0 and sigma PD),
so  x^T V Ninv V^T x = sum_pos y_i^2 - y_neg^2  with y = W_c^T x,
W_c = sqrt(alpha/|lam|) V U_eigvecs.  The class-independent quadratic
alpha x^T Binv x + REG x^T x = ||LA^T x||^2 via Cholesky (LA lower tri, so
col block j only needs k-blocks >= j -> 10/16 of the DMA+matmul).

Device work per core (queries sharded 8x256): ONE bf16 GEMM
[256,512]x[512,1664-ish] + squares (scalar engine), segmented reduce
(vector), tiny combine chain (gpsimd) and Ln (scalar).  cc is folded into
the GEMM via a rank-1 ones-matmul into the linW PSUM region.
"""
import math
from contextlib import ExitStack

import numpy as np
import ml_dtypes

import concourse.bass as bass
import concourse.tile as tile
from concourse import bacc, mybir
from concourse.bass_utils import run_bass_kernel_spmd

REG = 0.1
D = 512
C = 64
Q = 2048
N_CORES = 8
QC = Q // N_CORES          # 256 queries per core
P = 128                    # partitions
KT = D // P                # 4 contraction tiles
F32 = mybir.dt.float32
BF16 = mybir.dt.bfloat16
BF = ml_dtypes.bfloat16

# per-k column layout inside w_all: [xqt 256 | pos 1024 | neg 64 | lin 64 | la (k+1)*128]
XQT, POS, NEGL = QC, C * 16, 128
K_COLS = [XQT + POS + NEGL + (k + 1) * P for k in range(KT)]
K_BASE = [sum(K_COLS[:k]) for k in range(KT)]
W_COLS = sum(K_COLS)


# ---------------------------------------------------------------- host prep
def _prep(X_support, labels, X_query, m, kappa, nu, triu_diag, triu_lower,
          n_classes):
    f = np.float64
    Xs = np.asarray(X_support, f)
    Nn, Dd = Xs.shape
    Cc = int(n_classes)
    S = Nn // Cc
    m_ = np.asarray(m, f).reshape(1, Dd)
    kap = abs(float(kappa)) + 1e-6
    nu_ = max(float(nu), Dd - 1 + 1e-6)

    order = np.argsort(np.asarray(labels), kind="stable")
    Xg = Xs[order].reshape(Cc, S, Dd)
    mu = (kap / (kap + S)) * m_ + (S / (kap + S)) * Xg.mean(axis=1)  # [C,D]

    Lmask = np.tril(np.ones((Dd, Dd), f), -1)
    L = np.diag(np.abs(np.asarray(triu_diag, f))) + np.asarray(triu_lower, f) * Lmask
    B = L @ L.T + kap * (m_.T @ m_)
    coef = (kap + S + 1.0) / ((nu_ + S - Dd + 1.0) * (kap + S))
    alpha = (1.0 - REG) / coef
    common = nu_ + S + 1.0 - Dd
    beta = 0.5 * (common + Dd)

    Binv = np.linalg.inv(B)
    _, ldB = np.linalg.slogdet(B)

    U = np.concatenate([Xg.transpose(0, 2, 1), mu[:, :, None]], axis=2)  # [C,D,r]
    V = np.matmul(Binv, U)                                   # [C,D,r]
    Jinv = np.diag(np.concatenate([np.ones(S), [-1.0 / (kap + S)]]))
    M = Jinv[None] + np.swapaxes(U, 1, 2) @ V                # [C,r,r]

    lam, Uv = np.linalg.eigh(M)                              # ascending
    assert (lam[:, 0] < 0).all() and (lam[:, 1:] > 0).all(), "inertia != (1 neg)"
    Wp = np.einsum('cdr,crs->cds', V, Uv) * np.sqrt(alpha / np.abs(lam))[:, None, :]
    Wpos = Wp[:, :, 1:].transpose(1, 0, 2).reshape(Dd, Cc * S)   # [D, C*16]
    Wneg = Wp[:, :, 0].T                                     # [D... no: [C,D].T -> [D,C]

    Ninv = np.linalg.inv(M)
    _, ldM = np.linalg.slogdet(M)
    muB = mu @ Binv
    b = np.einsum("cdr,cd->cr", V, mu)
    kq = np.einsum("cd,cd->c", mu, muB)
    VN = V @ Ninv
    VNb = np.einsum("cdr,cr->cd", VN, b)
    Nb = np.einsum("crs,cs->cr", Ninv, b)

    linW = (-2.0 * alpha * (muB - VNb) - 2.0 * REG * mu).T   # [D,C]
    cc = (alpha * (kq - np.einsum("cr,cr->c", b, Nb))
          + REG * np.einsum("cd,cd->c", mu, mu) + common)    # [C]

    logdet = Dd * np.log(coef) + ldB + np.log(kap + S) + ldM
    bias = (math.lgamma(0.5 * (common + Dd)) - math.lgamma(0.5 * common)
            - 0.5 * Dd * np.log(common) - 0.5 * logdet)
    gam = bias + beta * np.log(common)                       # [C]

    LA = np.linalg.cholesky(alpha * Binv + REG * np.eye(Dd))  # lower tri

    # shared (per-k) weight blocks, bf16
    shared = []
    for k in range(KT):
        rows = slice(k * P, (k + 1) * P)
        blk = np.concatenate([Wpos[rows], Wneg[rows], linW[rows],
                              LA[rows, :(k + 1) * P]], axis=1)
        shared.append(blk.astype(BF))
    return (shared, cc.astype(np.float32), gam.astype(np.float32),
            float(alpha), float(beta))


# ---------------------------------------------------------------- device IR
_CACHE = {}


def _build(beta):
    nc = bacc.Bacc("TRN2", target_bir_lowering=False, debug=False,
                   num_devices=N_CORES)
    w_all = nc.declare_dram_parameter("w_all", [P, W_COLS], BF16, isOutput=False)
    gamg = nc.declare_dram_parameter("gamg", [P, C], F32, isOutput=False)
    ccr = nc.declare_dram_parameter("ccr", [1, C], BF16, isOutput=False)
    out = nc.declare_dram_parameter("out", [QC, C], F32, isOutput=True)
    ov = out[:].rearrange("(t p) c -> t p c", p=P)
    QT = QC // P

    with tile.TileContext(nc) as tc, ExitStack() as ctx:
        wpool = ctx.enter_context(tc.tile_pool(name="w", bufs=1))
        spool = ctx.enter_context(tc.tile_pool(name="s", bufs=2))
        pspool = ctx.enter_context(tc.tile_pool(name="ps", bufs=2, space="PSUM"))
        pppool = ctx.enter_context(tc.tile_pool(name="pp", bufs=4, space="PSUM"))

        # --- DMAs (issue order = arrival order): tiny consts, xqt, negla, pos
        ones_sb = wpool.tile([1, P], BF16, tag="ones")
        nc.vector.memset(ones_sb[:], 1.0)
        cc_sb = wpool.tile([1, C], BF16, tag="ccr")
        nc.sync.dma_start(cc_sb[:], ccr[:])
        gm_sb = wpool.tile([P, C], F32, tag="gam")
        nc.sync.dma_start(gm_sb[:], gamg[:])

        xqt_sb, negla_sb, pos_sb = [], [], []
        for k in range(KT):
            t_ = wpool.tile([P, XQT], BF16, tag=f"xqt{k}")
            nc.sync.dma_start(t_[:], w_all[:, K_BASE[k]:K_BASE[k] + XQT])
            xqt_sb.append(t_)
        for k in range(KT):
            ncols = NEGL + (k + 1) * P
            t_ = wpool.tile([P, ncols], BF16, tag=f"ngl{k}")
            o = K_BASE[k] + XQT + POS
            nc.sync.dma_start(t_[:], w_all[:, o:o + ncols])
            negla_sb.append(t_)
        for h in range(2):
            cur = []
            for k in range(KT):
                t_ = wpool.tile([P, 512], BF16, tag=f"pos{h}{k}")
                o = K_BASE[k] + XQT + h * 512
                nc.sync.dma_start(t_[:], w_all[:, o:o + 512])
                cur.append(t_)
            pos_sb.append(cur)

        # --- PE warmup: garbage matmuls on xqt tile 0 to ramp the clock
        ps_warm = pspool.tile([P, XQT], F32, tag="la")
        for _ in range(2):
            nc.tensor.matmul(ps_warm[:], xqt_sb[0][:, 0:P], xqt_sb[0][:],
                             start=True, stop=True)

        # --- per query-tile state
        negsq = [spool.tile([P, C], F32, tag="negsq", name=f"negsq{t}") for t in range(QT)]
        acc = [spool.tile([P, 1], F32, tag="acc", name=f"acc{t}") for t in range(QT)]
        dead = [spool.tile([P, 512], BF16, tag="dead", name=f"dead{t}") for t in range(QT)]
        sq = [spool.tile([P, POS], BF16, tag="sq", name=f"sq{t}") for t in range(QT)]
        segpos = [spool.tile([P, C], F32, tag="segp", name=f"segp{t}") for t in range(QT)]
        ps_negl = [None] * QT
        ps_la = [None] * QT

        # --- negl chunk: [neg 64 | lin 64], accumulate k, then +cc rank-1
        for t in range(QT):
            ps = pspool.tile([P, NEGL], F32, tag="negl")
            ps_negl[t] = ps
            for k in range(KT):
                nc.tensor.matmul(ps[:], xqt_sb[k][:, t * P:(t + 1) * P],
                                 negla_sb[k][:, 0:NEGL],
                                 start=(k == 0), stop=False)
            nc.tensor.matmul(ps[:, C:NEGL], ones_sb[:, 0:P], cc_sb[:],
                             start=False, stop=True, skip_group_check=True)
            nc.scalar.activation(negsq[t][:], ps[:, 0:C],
                                 mybir.ActivationFunctionType.Square)

        # --- la chunk: triangular [128,512], col block j needs k >= j
        for t in range(QT):
            ps = pspool.tile([P, 512], F32, tag="la")
            ps_la[t] = ps
            for j in range(KT):
                for k in range(j, KT):
                    nc.tensor.matmul(
                        ps[:, j * P:(j + 1) * P],
                        xqt_sb[k][:, t * P:(t + 1) * P],
                        negla_sb[k][:, NEGL + j * P:NEGL + (j + 1) * P],
                        start=(k == j), stop=(k == KT - 1),
                        skip_group_check=True)
            nc.scalar.activation(dead[t][:], ps[:],
                                 mybir.ActivationFunctionType.Square,
                                 accum_out=acc[t][:])

        # --- pos chunks (the 2x[512] halves), square + segmented reduce
        for h in range(2):
            for t in range(QT):
                ps = pppool.tile([P, 512], F32, tag="pos")
                for k in range(KT):
                    nc.tensor.matmul(ps[:], xqt_sb[k][:, t * P:(t + 1) * P],
                                     pos_sb[h][k][:],
                                     start=(k == 0), stop=(k == KT - 1))
                nc.scalar.activation(sq[t][:, h * 512:(h + 1) * 512], ps[:],
                                     mybir.ActivationFunctionType.Square)
                nc.vector.tensor_reduce(
                    out=segpos[t][:, h * 32:(h + 1) * 32],
                    in_=sq[t][:, h * 512:(h + 1) * 512].rearrange(
                        "p (c r) -> p c r", r=16),
                    axis=mybir.AxisListType.X, op=mybir.AluOpType.add)
                if h == 1:
                    # combine: td = (negsq+acc) - segpos + (lin+cc)
                    t1 = spool.tile([P, C], F32, tag="t1")
                    nc.vector.scalar_tensor_tensor(
                        out=t1[:], in0=negsq[t][:], scalar=acc[t][:],
                        in1=segpos[t][:], op0=mybir.AluOpType.add,
                        op1=mybir.AluOpType.subtract)
                    td = spool.tile([P, C], F32, tag="td")
                    nc.vector.tensor_add(td[:], t1[:], ps_negl[t][:, C:NEGL])
                    lg = spool.tile([P, C], F32, tag="lg")
                    nc.scalar.activation(lg[:], td[:],
                                         mybir.ActivationFunctionType.Ln)
                    res = spool.tile([P, C], F32, tag="res")
                    nc.vector.scalar_tensor_tensor(
                        out=res[:], in0=lg[:], scalar=-beta, in1=gm_sb[:],
                        op0=mybir.AluOpType.mult, op1=mybir.AluOpType.add)
                    nc.sync.dma_start(ov[t], res[:])

    nc.compile()
    return nc


def _get_nc(beta):
    key = round(beta, 9)
    if key not in _CACHE:
        _CACHE.clear()
        _CACHE[key] = _build(beta)
    return _CACHE[key]


def _in_maps(inputs_prepped, X_query):
    shared, cc, gam, alpha, beta = inputs_prepped
    gamg = np.ascontiguousarray(np.broadcast_to(gam[None, :], (P, C)))
    ccr = cc.reshape(1, C).astype(BF)
    Xq = np.asarray(X_query, np.float32)
    in_maps = []
    for i in range(N_CORES):
        xqt = Xq[i * QC:(i + 1) * QC].T.astype(BF)   # [D, QC]
        parts = []
        for k in range(KT):
            parts.append(np.ascontiguousarray(xqt[k * P:(k + 1) * P]))
            parts.append(shared[k])
        w = np.concatenate(parts, axis=1)
        assert w.shape == (P, W_COLS), w.shape
        in_maps.append({"w_all": np.ascontiguousarray(w),
                        "gamg": gamg, "ccr": ccr})
    return in_maps, beta


def kernel(X_support, labels, X_query, m, kappa, nu, triu_diag, triu_lower,
           n_classes):
    prepped = _prep(X_support, labels, X_query, m, kappa, nu, triu_diag,
                    triu_lower, n_classes)
    in_maps, beta = _in_maps(prepped, X_query)
    nc = _get_nc(beta)
    res = run_bass_kernel_spmd(nc, in_maps, list(range(N_CORES)))
    return np.concatenate([res.results[i]["out"] for i in range(N_CORES)],
                          axis=0)
